# revision 15
# baseline (speedup 1.0000x reference)
"""Two-layer GAT (graph attention network) on 8 Trainium2 NeuronCores.

v5 strategy (sharding per spec hint: dst-nodes + incoming edges partitioned
across cores, weights replicated; node tables replicated via host between
launches):

  * Nodes are snake-dealt to the 8 cores by in-degree (balanced degree
    profiles), then each core's 6250 nodes are grouped deg-descending into
    49 groups of 128 "slots" (partition = slot).
  * Dense phases are node-sharded matmuls with host-fused weights
    W_aug = [W | W@a_src | W@a_dst] so one matmul yields z, es, ed.
  * Edge phase gathers each edge's [z|es] source row with the batched
    GPSIMD dma_gather instruction (InstDMAGatherAnt): 1024 indices per
    instruction (the 16KB SWDGE descriptor-ring cap), rotated over 4 SWDGE
    queues -- measured ~2.3ns/edge vs ~11ns/edge for per-128-edge
    indirect DMAs.  Tables are f16 with 512B (L1) / 256B (L2) rows; int16
    gather indices address two half-tables (rows 0..32767 and the rest)
    via base-offset views, with an all-zero / es=-1e4 dummy row per half
    backing padding edges (their attention weight underflows to 0).
  * Layout is slot-major: gather column k of group g holds the k-th
    incoming edge of every slot.  Per "span" of groups (column budget
    ~72) the column counts are equalized so every DVE/Act op covers the
    whole span with one 3-dim access pattern.
  * w = exp(leakyrelu(es_src + ed_dst)) is computed on the Scalar engine
    whose activation broadcasts w across each head's D message columns in
    the same pass (stride-0 input AP); the f16 message multiply and the
    ragged column tree-sum run on DVE 2-byte packed APs (2-4x mode).
    Softmax max-subtraction is skipped (logits are O(1), exp cannot
    overflow f16 range; result is mathematically identical).
  * Layer-2's dense phase is fused into the layer-1 edge launch: each
    group's h1 block is transposed on the PE (identity matmul) and
    multiplied by W2_aug, so only 3 launches total (dense1, edge1+dense2,
    edge2).  ed tables and output rows are host-permuted into/out of
    slot order (graph-static node-level permutations), eliminating all
    scatter/indirect DMAs.
"""

import os
import sys

import numpy as np

for _p in ("/opt/trn_rl_repo", "/root/.axon_site/_ro/trn_rl_repo"):
    if os.path.isdir(_p) and _p not in sys.path:
        sys.path.insert(0, _p)

# ---------------------------------------------------------------- constants
N = 50000
E = 800000
IN_DIM = 128
HID = 16
HEADS = 8
OUT_DIM = 32
NEG_SLOPE = 0.2

CORES = 8
NPC = N // CORES
P = 128
DENSE_W = 144             # dense1 out row: z(128) | es(8) | ed(8)
NT = (NPC + P - 1) // P
ES_PAD = -1.0e4

# v5 table geometry: per-core COMPACTED gather table (only the ~43k source
# rows the core references, plus dummy rows 0 and TR-1), accessed through
# two OVERLAPPING int16-addressable views: view A = rows [0, 32768), view
# B = rows [TR-32768, TR).  Rows in the overlap (~52%) can be gathered
# from either view, letting the planner balance each slot's A/B column
# split down to ~max-degree packing.
HALF0 = 32768             # rows per view
G_TOT = (NPC + P - 1) // P          # 49 groups per core
NSLOT = G_TOT * P                   # 6272 slots per core
SPAN_COLS_BUDGET = 72     # max equalized columns per span (L1: x512B/part)
GMAX = 1024               # max indices per dma_gather (SWDGE ring cap)
NQ = 4                    # SWDGE queues
D2W = OUT_DIM + 2         # fused dense2 out row: z2(32) | es2 | ed2

_EDGE_PLAN_CACHE = {}


# ---------------------------------------------------------------- host prep
def fuse_weights(W, a_src, a_dst, H, D, width):
    """W:[K, H*D] -> [K, width] = [W | W@a_src | W@a_dst] (f32)."""
    K = W.shape[0]
    Wr = W.reshape(K, H, D)
    wes = np.einsum("khd,hd->kh", Wr, a_src)
    wed = np.einsum("khd,hd->kh", Wr, a_dst)
    out = np.zeros((K, width), dtype=np.float32)
    out[:, : H * D] = W
    out[:, H * D : H * D + H] = wes
    out[:, H * D + H : H * D + 2 * H] = wed
    return out


def _bass_mods():
    import concourse.bass as bass
    import concourse.tile as tile
    from concourse import mybir

    return bass, tile, mybir


_SAFE_TC = None


def _safe_tile_context():
    """TileContext whose kernel-tail drain never carries more than 1 sem
    wait per instruction (this container's walrus rejects multi-sync-wait
    CTRL instructions); excess waits are moved onto preceding nops."""
    global _SAFE_TC
    if _SAFE_TC is not None:
        return _SAFE_TC
    import concourse.tile as tile
    from concourse import mybir
    from concourse.vector_clock import ScopedClock

    class TileContextSafe(tile.TileContext):
        def _add_instruction(self, inst):
            si = inst.sync_info
            if (
                si is not None
                and si.on_wait
                and len(si.on_wait) > 1
                and inst.engine != mybir.EngineType.Unassigned
            ):
                waits = list(si.on_wait)
                si.on_wait = waits[-1:]
                for w in waits[:-1]:
                    nop = mybir.InstNoOp(
                        name=self.nc.get_next_instruction_name(), ins=[], outs=[]
                    )
                    nop.engine = inst.engine
                    nop.sync_info = mybir.SyncInfo(on_wait=[w], on_update=[])
                    super()._add_instruction(nop)
            super()._add_instruction(inst)

        def _drain_and_barrier(self, tick_clock, wait_clock):
            nc = self.nc
            nops = [nc.sync.nop(nofuse=True) for _ in range(28)]
            drain_inst = nc.sync.drain()
            wait_clock.add_sem_waits(
                drain_inst.ins, ScopedClock({None: tick_clock.global_clock})
            )
            si = drain_inst.ins.sync_info
            waits = list(si.on_wait) if si is not None and si.on_wait else []
            if len(waits) > 1:
                si.on_wait = waits[:1]
                rest = waits[1:]
                assert len(rest) <= len(nops), "raise nop count"
                for k, w in enumerate(rest):
                    nops[k].ins.sync_info = mybir.SyncInfo(
                        on_wait=[w], on_update=[]
                    )

            nc.all_engine_barrier()
            assert self.sems is not None
            popped = nc._tile_sem_poison_stack.pop()
            assert popped is self._sem_poison
            nc.clear_and_free_semaphores(list(self.sems.allocated().values()))
            nc.all_engine_barrier()

    _SAFE_TC = TileContextSafe
    return _SAFE_TC


def _ap(tile_ap, col_off, dims):
    """Custom free-dim AP on an SBUF tile: keep the tile's partition dim,
    replace free dims with [step, count] pairs (steps in elements)."""
    import concourse.bass as bass

    part = list(tile_ap.ap[0])
    return bass.AP(
        tile_ap.tensor,
        tile_ap.offset + col_off,
        [part] + [list(d) for d in dims],
    )


# ---------------------------------------------------------------- v5 plan
def build_plan_v5(src, dst):
    """Snake-deal nodes to cores, group deg-descending, equalize gather
    column counts per span, and emit per-core wrapped int16 index arrays.

    Returns dict with:
      spans:  list of (G_s, K0s, K1s)   shared across cores
      chunks: list per span of (half, colbase, n_idx, s_off) gather chunks
      S_tot:  total idx columns (int16) per core
      cores:  per-core dict: idx [128, S_tot] i16, node_of [NSLOT] i64
      slot_of/core_of: [N] maps
    """
    key = ("v5", src.tobytes(), dst.tobytes())
    h = hash(key)
    if h in _EDGE_PLAN_CACHE:
        return _EDGE_PLAN_CACHE[h]

    deg = np.bincount(dst, minlength=N).astype(np.int64)
    order = np.argsort(-deg, kind="stable")
    # snake pattern over 16-rank cycles: 0..7,7..0
    pat = np.concatenate([np.arange(CORES), np.arange(CORES)[::-1]])
    core_of_rank = pat[np.arange(N) % 16]
    node_core = np.empty(N, dtype=np.int64)
    node_core[order] = core_of_rank
    # slot within core: nodes in rank order
    slot_of = np.empty(N, dtype=np.int64)
    node_of = np.full((CORES, NSLOT), -1, dtype=np.int64)
    for c in range(CORES):
        nodes_c = order[core_of_rank == c]   # deg-descending
        slot_of[nodes_c] = np.arange(len(nodes_c))
        node_of[c, : len(nodes_c)] = nodes_c

    e_core = node_core[dst]
    # per-core compacted table: sorted unique referenced source nodes
    used = []
    for c in range(CORES):
        used.append(np.unique(src[e_core == c].astype(np.int64)))
    # 128 rotated dummy rows at each end (a single hot dummy row serializes
    # on one DRAM bank); real nodes at compact rows 128..TR-129
    TR = max(len(u) for u in used) + 2 * P
    BA = TR - HALF0                              # view B base row
    assert TR <= 2 * HALF0, "compact table exceeds two int16 views"

    # per-core compact position of each edge's source (P..P+len-1)
    cpos_of = np.zeros((CORES, N), dtype=np.int64)
    for c in range(CORES):
        cpos_of[c, used[c]] = P + np.arange(len(used[c]))
    cpos_e = cpos_of[e_core, src]                # [E]

    # view category per edge: A-only (<BA) / flexible / B-only (>=32768)
    cat = np.where(cpos_e < BA, 0, np.where(cpos_e < HALF0, 1, 2))
    na = np.bincount(dst[cat == 0], minlength=N).astype(np.int64)
    nb = np.bincount(dst[cat == 2], minlength=N).astype(np.int64)

    # re-sort slots within each core by the node's true column requirement
    # max(2*na, 2*nb, deg) -- groups become tight in the binding constraint
    cost = np.maximum(np.maximum(2 * na, 2 * nb), deg)
    for c in range(CORES):
        nodes_c = node_of[c][node_of[c] >= 0]
        o = np.argsort(cost[nodes_c], kind="stable")[::-1]
        nodes_c = nodes_c[o]
        node_of[c] = -1
        node_of[c, : len(nodes_c)] = nodes_c
        slot_of[nodes_c] = np.arange(len(nodes_c))

    # group-level limits (max over cores) -> span-level K0s/K1s
    def slotted(vals):
        out = np.zeros((CORES, NSLOT), dtype=np.int64)
        for c in range(CORES):
            m = node_of[c] >= 0
            out[c, m] = vals[node_of[c, m]]
        return out.reshape(CORES, G_TOT, P)

    L0_g = slotted(na).max(axis=(0, 2))
    L1_g = slotted(nb).max(axis=(0, 2))
    LD_g = slotted(deg).max(axis=(0, 2))

    # greedy span packing (groups are deg-descending so cost is ~monotone)
    def span_k(gl):
        L0 = int(max(L0_g[t] for t in gl))
        L1 = int(max(L1_g[t] for t in gl))
        LD = int(max(LD_g[t] for t in gl))
        S = max(LD, L0 + L1, 2)
        k0 = max(L0, S - L1, 1)
        return k0, max(S - k0, 1)

    spans = []
    cur = []
    for g in range(G_TOT):
        trial = cur + [g]
        k0, k1 = span_k(trial)
        if cur and len(trial) * (k0 + k1) > SPAN_COLS_BUDGET:
            spans.append(cur)
            cur = [g]
        else:
            cur = trial
    if cur:
        spans.append(cur)
    span_meta = [(len(gl),) + span_k(gl) for gl in spans]

    # per-node A-side count: K1s of the node's span bounds the B side
    K1s_of_g = np.empty(G_TOT, dtype=np.int64)
    for gl, (_, k0, k1) in zip(spans, span_meta):
        for t in gl:
            K1s_of_g[t] = k1
    nA = np.maximum(na, deg - K1s_of_g[slot_of // P])

    # chunk layout (shared across cores)
    chunks = []       # per span: list of (half, colbase, n_idx, s_off)
    s_off = 0
    for (G_s, K0s, K1s) in span_meta:
        ch = []
        for half, ncols, colbase in ((0, G_s * K0s, 0), (1, G_s * K1s, G_s * K0s)):
            nidx = ncols * P
            done = 0
            while done < nidx:
                n = min(GMAX, nidx - done)
                ch.append((half, colbase + done // P, n, s_off))
                s_off += n // 16
                done += n
        chunks.append(ch)
    S_tot = s_off

    # per-edge placement: sort by (dst, category); first nA[dst] edges of
    # each node go to the A region (A-only first, then flexible), rest to B
    ord2 = np.lexsort((cat, dst))
    scpos = cpos_e[ord2]
    sdst = dst[ord2]
    starts = np.zeros(N + 1, dtype=np.int64)
    np.cumsum(deg, out=starts[1:])
    rank = np.arange(E, dtype=np.int64) - starts[sdst]
    sis1 = rank >= nA[sdst]
    col = np.where(sis1, rank - nA[sdst], rank)

    K0max = int(max(k0 for (_, k0, _) in span_meta))
    K1max = int(max(k1 for (_, _, k1) in span_meta))
    se_core = e_core[ord2]
    e_slot = slot_of[sdst]
    e_g = e_slot // P
    e_p = e_slot % P

    plan_cores = []
    for c in range(CORES):
        m = se_core == c
        # rotated dummies: pad desc for slot p reads dummy row p (view A)
        # or row TR-P+p (view B, local HALF0-P+p)
        A = np.broadcast_to(
            np.arange(P, dtype=np.int32), (G_TOT, K0max, P)
        ).copy()
        B = np.broadcast_to(
            np.arange(HALF0 - P, HALF0, dtype=np.int32), (G_TOT, K1max, P)
        ).copy()
        m0 = m & ~sis1
        m1 = m & sis1
        A[e_g[m0], col[m0], e_p[m0]] = scpos[m0]
        B[e_g[m1], col[m1], e_p[m1]] = scpos[m1] - BA
        # build wrapped idx stream per chunk layout
        idx = np.zeros((16, S_tot), dtype=np.int16)
        g_base = 0
        for si_, (G_s, K0s, K1s) in enumerate(span_meta):
            for (half, colbase, n, so) in chunks[si_]:
                src_arr = A if half == 0 else B
                Ks = K0s if half == 0 else K1s
                cb = colbase if half == 0 else colbase - G_s * K0s
                ncols = n // P
                qs = cb + np.arange(ncols)
                vals = src_arr[g_base + qs // Ks, qs % Ks, :]  # [ncols, P]
                stream = vals.reshape(-1).astype(np.int16)     # pos = q*128+p
                idx[:, so : so + n // 16] = stream.reshape(n // 16, 16).T
            g_base += G_s
        plan_cores.append(
            {
                "idx": np.tile(idx, (8, 1)),
                "node_of": node_of[c],
                "used": used[c],
            }
        )

    plan = {
        "spans": span_meta,
        "chunks": chunks,
        "S_tot": S_tot,
        "TR": TR,
        "cores": plan_cores,
        "ndesc": sum(
            G_s * (K0s + K1s) * P for (G_s, K0s, K1s) in span_meta
        ),
    }
    _EDGE_PLAN_CACHE[h] = plan
    return plan


# ---------------------------------------------------------------- builders
def build_dense_nc():
    """Per-core dense1: out[tile] = xT[:, tile].T @ Waug -> [NT*P, 144]."""
    bass, tile, mybir = _bass_mods()
    f32 = mybir.dt.float32
    nc = bass.Bass("TRN2")
    xT = nc.dram_tensor("xt", [P, NT * P], f32, kind="ExternalInput")
    W = nc.dram_tensor("waug", [P, DENSE_W], f32, kind="ExternalInput")
    OUTD = nc.dram_tensor("outd", [NT * P, DENSE_W], f32, kind="ExternalOutput")

    with _safe_tile_context()(nc) as tc:
        from contextlib import ExitStack

        with ExitStack() as ctx:
            const = ctx.enter_context(tc.tile_pool(name="const", bufs=1))
            work = ctx.enter_context(tc.tile_pool(name="work", bufs=3))
            psum = ctx.enter_context(tc.tile_pool(name="psum", bufs=4, space="PSUM"))

            wsb = const.tile([P, DENSE_W], f32)
            nc.sync.dma_start(out=wsb[:], in_=W[:, :])
            xsb = const.tile([P, NT * P], f32)
            nc.sync.dma_start(out=xsb[:], in_=xT[:, :])

            for t in range(NT):
                ps = psum.tile([P, DENSE_W], f32, tag="ps")
                nc.tensor.matmul(
                    out=ps[:],
                    lhsT=xsb[:, t * P : (t + 1) * P],
                    rhs=wsb[:],
                    start=True,
                    stop=True,
                )
                st = work.tile([P, DENSE_W], f32, tag="st")
                nc.vector.tensor_copy(out=st[:], in_=ps[:])
                nc.sync.dma_start(out=OUTD[t * P : (t + 1) * P, :], in_=st[:])
    return nc


def build_edge_nc_v5(spans, chunks, S_tot, TR, RWp, ES_OFF, H, D, elu, fuse_w2):
    """v5 edge phase for one GAT layer (per core, SPMD-identical).

    T [TROWS, RWp] f16 gather table ([z|es|pad] rows, dummy rows 0 / N+1).
    IDX [128, S_tot] i16 wrapped gather indices.  ED [NSLOT, H] f32
    slot-ordered ed values.  Output: OUT [NSLOT, ZW] f32 (plain layer) or
    D2O [NSLOT, D2W] f16 (fused dense2, needs W2A [128, D2W] f16 + IDN).
    """
    bass, tile, mybir = _bass_mods()
    from contextlib import ExitStack
    from concourse import library_config, library_overlay

    f16 = mybir.dt.float16
    f32 = mybir.dt.float32
    i16 = mybir.dt.int16
    ZW = H * D
    colsmax = max(G_s * (K0s + K1s) for (G_s, K0s, K1s) in spans)
    gmax = max(G_s for (G_s, _, _) in spans)

    nc = bass.Bass("TRN2", num_swdge_queues=NQ)
    T = nc.dram_tensor("tbl", [TR, RWp], f16, kind="ExternalInput")
    IDX = nc.dram_tensor("idx", [P, S_tot], i16, kind="ExternalInput")
    ED = nc.dram_tensor("edt", [NSLOT, H], f32, kind="ExternalInput")
    if fuse_w2:
        W2A = nc.dram_tensor("w2a", [P, D2W], f16, kind="ExternalInput")
        IDN = nc.dram_tensor("idn", [P, P], f16, kind="ExternalInput")
        D2O = nc.dram_tensor("d2o", [NSLOT, D2W], f16, kind="ExternalOutput")
    else:
        OUT = nc.dram_tensor("out", [NSLOT, ZW], f32, kind="ExternalOutput")

    with _safe_tile_context()(nc) as tc:
        with ExitStack() as ctx:
            nc.gpsimd.load_library(library_config.mlp)
            regs = {}

            def reg_of(n):
                if n not in regs:
                    regs[n] = nc.gpsimd.to_reg(n)
                return regs[n]

            const = ctx.enter_context(tc.tile_pool(name="const", bufs=1))
            idxp = ctx.enter_context(tc.tile_pool(name="idxp", bufs=3))
            gath = ctx.enter_context(tc.tile_pool(name="gath", bufs=3))
            wrk = ctx.enter_context(tc.tile_pool(name="wrk", bufs=2))
            sml = ctx.enter_context(tc.tile_pool(name="sml", bufs=2))
            if fuse_w2:
                psum = ctx.enter_context(
                    tc.tile_pool(name="psum", bufs=4, space="PSUM")
                )
                w2sb = const.tile([P, D2W], f16)
                nc.sync.dma_start(out=w2sb[:], in_=W2A[:, :])
                idn = const.tile([P, P], f16)
                nc.sync.dma_start(out=idn[:], in_=IDN[:, :])

            qn = [0]
            g_base = 0
            for s, (G_s, K0s, K1s) in enumerate(spans):
                colsA = G_s * K0s
                colsT = G_s * (K0s + K1s)
                ch = chunks[s]
                S_span = sum(n // 16 for (_, _, n, _) in ch)
                so0 = ch[0][3]

                idxt = idxp.tile([P, S_span], i16, tag="idx")
                nc.sync.dma_start(out=idxt[:], in_=IDX[:, so0 : so0 + S_span])

                gt = gath.tile([P, colsmax * RWp], f16, tag="gt")
                for (half, colbase, n, so) in ch:
                    in_ap = T[0:HALF0, :] if half == 0 else T[TR - HALF0 : TR, :]
                    nc.gpsimd.dma_gather(
                        out_ap=_ap(gt[:], colbase * RWp, [[RWp, n // P], [1, RWp]]),
                        in_ap=in_ap,
                        idxs_ap=idxt[:, so - so0 : so - so0 + n // 16],
                        num_idxs=n,
                        num_idxs_reg=reg_of(n),
                        elem_size=RWp,
                        queue_num=qn[0] % NQ,
                    )
                    qn[0] += 1

                edt = sml.tile([P, gmax * H], f32, tag="ed")
                nc.sync.dma_start(
                    out=_ap(edt[:], 0, [[H, G_s], [1, H]]),
                    in_=bass.AP(
                        ED, g_base * P * H, [[H, P], [P * H, G_s], [1, H]]
                    ),
                )

                # ---- logits e = es_src + ed_dst (f32), then lrelu
                e_t = sml.tile([P, colsmax * H], f32, tag="e")
                nc.vector.tensor_tensor(
                    out=_ap(e_t[:], 0, [[K0s * H, G_s], [H, K0s], [1, H]]),
                    in0=_ap(gt[:], ES_OFF, [[K0s * RWp, G_s], [RWp, K0s], [1, H]]),
                    in1=_ap(edt[:], 0, [[H, G_s], [0, K0s], [1, H]]),
                    op=mybir.AluOpType.add,
                )
                nc.vector.tensor_tensor(
                    out=_ap(e_t[:], colsA * H, [[K1s * H, G_s], [H, K1s], [1, H]]),
                    in0=_ap(
                        gt[:],
                        colsA * RWp + ES_OFF,
                        [[K1s * RWp, G_s], [RWp, K1s], [1, H]],
                    ),
                    in1=_ap(edt[:], 0, [[H, G_s], [0, K1s], [1, H]]),
                    op=mybir.AluOpType.add,
                )
                t3 = sml.tile([P, colsmax * H], f32, tag="t3")
                nc.vector.scalar_tensor_tensor(
                    out=t3[:, : colsT * H],
                    in0=e_t[:, : colsT * H],
                    scalar=NEG_SLOPE,
                    in1=e_t[:, : colsT * H],
                    op0=mybir.AluOpType.mult,
                    op1=mybir.AluOpType.max,
                )
                # ---- w wide (broadcast over D) and w narrow, on Act engine
                wx = wrk.tile([P, colsmax * ZW], f16, tag="wx")
                nc.scalar.activation(
                    out=_ap(wx[:], 0, [[ZW, colsT], [D, H], [1, D]]),
                    in_=_ap(t3[:], 0, [[H, colsT], [1, H], [0, D]]),
                    func=mybir.ActivationFunctionType.Exp,
                )
                wn = sml.tile([P, colsmax * H], f16, tag="wn")
                nc.scalar.activation(
                    out=wn[:, : colsT * H],
                    in_=t3[:, : colsT * H],
                    func=mybir.ActivationFunctionType.Exp,
                )
                # ---- messages: gt.z *= wx  (in place, f16 packed)
                nc.vector.tensor_tensor(
                    out=_ap(gt[:], 0, [[RWp, colsT], [1, ZW]]),
                    in0=_ap(gt[:], 0, [[RWp, colsT], [1, ZW]]),
                    in1=_ap(wx[:], 0, [[ZW, colsT], [1, ZW]]),
                    op=mybir.AluOpType.mult,
                )

                # ---- ragged tree-sum of columns into column 0 (per group)
                def tree(base_off, K, width, stride, tilebuf):
                    kk = K
                    while kk > 1:
                        half_ = kk - kk // 2
                        nadd = kk - half_
                        nc.vector.tensor_tensor(
                            out=_ap(
                                tilebuf[:],
                                base_off,
                                [[K * stride, G_s], [stride, nadd], [1, width]],
                            ),
                            in0=_ap(
                                tilebuf[:],
                                base_off,
                                [[K * stride, G_s], [stride, nadd], [1, width]],
                            ),
                            in1=_ap(
                                tilebuf[:],
                                base_off + half_ * stride,
                                [[K * stride, G_s], [stride, nadd], [1, width]],
                            ),
                            op=mybir.AluOpType.add,
                        )
                        kk = half_

                tree(0, K0s, ZW, RWp, gt)
                tree(colsA * RWp, K1s, ZW, RWp, gt)
                nc.vector.tensor_tensor(
                    out=_ap(gt[:], 0, [[K0s * RWp, G_s], [1, ZW]]),
                    in0=_ap(gt[:], 0, [[K0s * RWp, G_s], [1, ZW]]),
                    in1=_ap(gt[:], colsA * RWp, [[K1s * RWp, G_s], [1, ZW]]),
                    op=mybir.AluOpType.add,
                )
                tree(0, K0s, H, H, wn)
                tree(colsA * H, K1s, H, H, wn)
                den = sml.tile([P, gmax * H], f32, tag="den")
                nc.vector.tensor_tensor(
                    out=_ap(den[:], 0, [[H, G_s], [1, H]]),
                    in0=_ap(wn[:], 0, [[K0s * H, G_s], [1, H]]),
                    in1=_ap(wn[:], colsA * H, [[K1s * H, G_s], [1, H]]),
                    op=mybir.AluOpType.add,
                )
                rs = sml.tile([P, gmax * H], f32, tag="rs")
                nc.vector.tensor_scalar_add(
                    rs[:, : G_s * H], den[:, : G_s * H], 1e-30
                )
                nc.vector.reciprocal(den[:, : G_s * H], rs[:, : G_s * H])
                # ---- out = num * recip (broadcast recip over D)
                ot = wrk.tile([P, gmax * ZW], f32, tag="ot")
                nc.vector.tensor_tensor(
                    out=_ap(ot[:], 0, [[ZW, G_s], [D, H], [1, D]]),
                    in0=_ap(gt[:], 0, [[K0s * RWp, G_s], [D, H], [1, D]]),
                    in1=_ap(den[:], 0, [[H, G_s], [1, H], [0, D]]),
                    op=mybir.AluOpType.mult,
                )
                if elu:
                    ngx = wrk.tile([P, gmax * ZW], f32, tag="ngx")
                    nc.vector.tensor_scalar_min(
                        ngx[:, : G_s * ZW], ot[:, : G_s * ZW], 0.0
                    )
                    ex = wrk.tile([P, gmax * ZW], f32, tag="ex")
                    nc.scalar.activation(
                        out=ex[:, : G_s * ZW],
                        in_=ngx[:, : G_s * ZW],
                        func=mybir.ActivationFunctionType.Exp,
                    )
                    pos = wrk.tile([P, gmax * ZW], f32, tag="pos")
                    nc.vector.tensor_scalar_max(
                        pos[:, : G_s * ZW], ot[:, : G_s * ZW], 0.0
                    )
                    h1 = wrk.tile([P, gmax * ZW], f16, tag="h1")
                    nc.vector.scalar_tensor_tensor(
                        out=h1[:, : G_s * ZW],
                        in0=ex[:, : G_s * ZW],
                        scalar=-1.0,
                        in1=pos[:, : G_s * ZW],
                        op0=mybir.AluOpType.add,
                        op1=mybir.AluOpType.add,
                    )
                if fuse_w2:
                    # dense2 per group: transpose h1_g on PE, then @ W2A
                    d2 = wrk.tile([P, gmax * D2W], f16, tag="d2")
                    for gi in range(G_s):
                        pst = psum.tile([P, P], f32, tag="pst")
                        nc.tensor.matmul(
                            out=pst[:],
                            lhsT=h1[:, gi * ZW : (gi + 1) * ZW],
                            rhs=idn[:],
                            start=True,
                            stop=True,
                        )
                        h1t = sml.tile([P, P], f16, tag="h1t")
                        nc.scalar.activation(
                            out=h1t[:],
                            in_=pst[:],
                            func=mybir.ActivationFunctionType.Copy,
                        )
                        ps2 = psum.tile([P, D2W], f32, tag="ps2")
                        nc.tensor.matmul(
                            out=ps2[:],
                            lhsT=h1t[:],
                            rhs=w2sb[:],
                            start=True,
                            stop=True,
                        )
                        nc.scalar.activation(
                            out=d2[:, gi * D2W : (gi + 1) * D2W],
                            in_=ps2[:],
                            func=mybir.ActivationFunctionType.Copy,
                        )
                    nc.sync.dma_start(
                        out=bass.AP(
                            D2O,
                            g_base * P * D2W,
                            [[D2W, P], [P * D2W, G_s], [1, D2W]],
                        ),
                        in_=_ap(d2[:], 0, [[D2W, G_s], [1, D2W]]),
                    )
                else:
                    nc.sync.dma_start(
                        out=bass.AP(
                            OUT,
                            g_base * P * ZW,
                            [[ZW, P], [P * ZW, G_s], [1, ZW]],
                        ),
                        in_=_ap(ot[:], 0, [[ZW, G_s], [1, ZW]]),
                    )
                g_base += G_s
    library_overlay.lower_extended_insts(nc)
    return nc


# ---------------------------------------------------------------- run layer
def _run_spmd(nc, in_maps, collect, label):
    from concourse.bass_utils import run_bass_kernel_spmd

    trace = bool(int(os.environ.get("GAT_TRACE", "0")))
    res = run_bass_kernel_spmd(
        nc, in_maps, core_ids=list(range(CORES)), trace=trace
    )
    if collect is not None:
        collect.append((label, getattr(res, "exec_time_ns", None)))
    return res.results


def _dense_phase(x, Waug, collect, label):
    xT = np.ascontiguousarray(x.T.astype(np.float32))
    xT_pad = np.zeros((P, NT * P), dtype=np.float32)
    in_maps = []
    for c in range(CORES):
        xc = np.array(xT_pad)
        xc[:, :NPC] = xT[:, c * NPC : (c + 1) * NPC]
        in_maps.append({"xt": xc, "waug": Waug})
    outs = _run_spmd(build_dense_nc(), in_maps, collect, label)
    return np.concatenate([o["outd"][:NPC] for o in outs], axis=0)


def _make_table(z, es, used, TR, RWp):
    """Per-core compacted f16 gather table [TR, RWp]: compact row P+i =
    [z|es] of node used[i]; rows 0..P-1 and TR-P..TR-1 are rotated dummy
    rows carrying es = ES_PAD (so padding edges get weight 0)."""
    ZW = z.shape[1]
    Hh = es.shape[1]
    t = np.zeros((TR, RWp), dtype=np.float16)
    L = len(used)
    t[P : P + L, :ZW] = z[used].astype(np.float16)
    t[P : P + L, ZW : ZW + Hh] = es[used].astype(np.float16)
    t[:P, ZW : ZW + Hh] = ES_PAD
    t[TR - P :, ZW : ZW + Hh] = ES_PAD
    return t


def _edge_phase_v5(z, es, ed, plan, RWp, H, D, elu, W2a_f16, collect, label):
    ZW = H * D
    nc = build_edge_nc_v5(
        plan["spans"], plan["chunks"], plan["S_tot"], plan["TR"], RWp, ZW,
        H, D, elu, fuse_w2=W2a_f16 is not None,
    )
    idn = np.eye(P, dtype=np.float16)
    in_maps = []
    for c in range(CORES):
        pc = plan["cores"][c]
        edc = np.zeros((NSLOT, H), dtype=np.float32)
        m = pc["node_of"] >= 0
        edc[m] = ed[pc["node_of"][m]].astype(np.float32)
        tbl = _make_table(z, es, pc["used"], plan["TR"], RWp)
        im = {"tbl": tbl, "idx": pc["idx"], "edt": edc}
        if W2a_f16 is not None:
            im["w2a"] = W2a_f16
            im["idn"] = idn
        in_maps.append(im)
    outs = _run_spmd(nc, in_maps, collect, label)
    W = D2W if W2a_f16 is not None else ZW
    key = "d2o" if W2a_f16 is not None else "out"
    full = np.zeros((N, W), dtype=np.float32)
    for c in range(CORES):
        pc = plan["cores"][c]
        m = pc["node_of"] >= 0
        full[pc["node_of"][m]] = outs[c][key][m].astype(np.float32)
    return full


# ---------------------------------------------------------------- kernel
def kernel(h, W1, a1_src, a1_dst, W2, a2_src, a2_dst, src, dst, _collect=None):
    h = np.asarray(h, dtype=np.float32)
    W1 = np.asarray(W1, dtype=np.float32)
    W2 = np.asarray(W2, dtype=np.float32)
    a1_src = np.asarray(a1_src, dtype=np.float32)
    a1_dst = np.asarray(a1_dst, dtype=np.float32)
    a2_src = np.asarray(a2_src, dtype=np.float32)
    a2_dst = np.asarray(a2_dst, dtype=np.float32)
    src = np.asarray(src)
    dst = np.asarray(dst)

    W1a = fuse_weights(W1, a1_src, a1_dst, HEADS, HID, DENSE_W)
    W2a = fuse_weights(W2, a2_src, a2_dst, 1, OUT_DIM, D2W).astype(np.float16)

    plan = build_plan_v5(src, dst)

    # launch 1: dense1 -> [N, 144] f32 (z1 | es1 | ed1)
    d1 = _dense_phase(h, W1a, _collect, "dense1")
    # launch 2: edge1 + fused dense2 -> [N, 34] (z2 | es2 | ed2)
    d2 = _edge_phase_v5(
        d1[:, :128], d1[:, 128:136], d1[:, 136:144], plan,
        RWp=256, H=HEADS, D=HID, elu=True, W2a_f16=W2a,
        collect=_collect, label="edge1",
    )
    # launch 3: edge2 -> [N, 32] f32
    out = _edge_phase_v5(
        d2[:, :OUT_DIM], d2[:, OUT_DIM : OUT_DIM + 1],
        d2[:, OUT_DIM + 1 : OUT_DIM + 2], plan,
        RWp=128, H=1, D=OUT_DIM, elu=False, W2a_f16=None,
        collect=_collect, label="edge2",
    )
    return out.astype(np.float32)
